# revision 5
# baseline (speedup 1.0000x reference)
"""nn_Cate3Classifier — 8-core Trainium2 Bass kernel (fp8 DoubleRow rewrite).

Math (see reference):
    h   = swem_vec @ W_fc (+ b_fc)        # b_fc cancels inside BatchNorm
    hn  = relu((h - mu) * rsqrt(var + eps) * gamma + beta)   # batch stats over ALL B rows
    out = hn @ W_clf + b_clf
    out[i, j] = -100 where mask2[cate2[i], j]

Distribution: pure data-parallel over the batch (2048 rows/core on 8 cores).
Weights/masks replicated. BN batch statistics use one AllReduce of per-core
[sum_h, sum_h^2] (f32 [128, 8] = 4 KiB), preceded by a tiny dummy AllReduce
at kernel start that absorbs inter-core start skew and warms the cc stream.

Key speed choices vs the bf16 baseline:
  - fc matmul runs in fp8 e4m3 DoubleRow perf mode (2 contraction rows per
    cycle -> 2x PE throughput). W_fc is pre-scaled by 64 host-side so its
    values sit in e4m3's normal range; the scale cancels inside BatchNorm
    (eps is scaled by 64^2 to keep the math exact).
  - x is pre-transposed AND pre-paired host-side, so all device loads are
    plain contiguous DMAs (no HW DMA-transpose stream, no phase fencing).
  - single late stats AllReduce; dummy AR at t=0 hides the entry barrier.
  - BN apply + clf matmul + mask + store pipelined per 512-row chunk with
    the BN work split across Scalar/Vector/GpSimd engines.
"""

import numpy as np
import ml_dtypes

B, D, H, C3, C2 = 16384, 2048, 512, 125, 64
NCORES = 8
BL = B // NCORES      # 2048 rows per core
KP = D // 256         # 8 fp8 DoubleRow contraction pairs (2x128 each)
RF = H // 128         # 4 feature chunks
NRC = BL // 512       # 4 row chunks of 512
NT = BL // 128        # 16 output row tiles of 128
BN_EPS = 1e-5
MASK_VAL = -100.0
WSCALE = 64.0         # host pre-scale on W_fc; cancels in BN (eps scaled too)

_CACHE = {}


def _build_nc():
    from contextlib import ExitStack

    import concourse.bass as bass
    import concourse.tile as tile
    from concourse import bacc, mybir

    f32 = mybir.dt.float32
    bf16 = mybir.dt.bfloat16
    fp8 = mybir.dt.float8e4
    i32 = mybir.dt.int32
    AF = mybir.ActivationFunctionType
    OP = mybir.AluOpType
    DR = mybir.MatmulPerfMode.DoubleRow

    nc = bacc.Bacc("TRN2", target_bir_lowering=False, debug=False, num_devices=NCORES)

    xq_d = nc.dram_tensor("xq", [KP, 128, 2 * BL], fp8, kind="ExternalInput")
    wfc_d = nc.dram_tensor("wfc", [128, KP * RF * 2 * 128], fp8, kind="ExternalInput")
    wclf_d = nc.dram_tensor("wclf", [128, RF * C3], bf16, kind="ExternalInput")
    bclf_d = nc.dram_tensor("bclf", [1, C3], bf16, kind="ExternalInput")
    gam_d = nc.dram_tensor("gam", [128, RF], f32, kind="ExternalInput")
    bet_d = nc.dram_tensor("bet", [128, RF], f32, kind="ExternalInput")
    m2_d = nc.dram_tensor("m2", [C2, C3], mybir.dt.uint8, kind="ExternalInput")
    cat_d = nc.dram_tensor("cat", [128, NT], i32, kind="ExternalInput")
    out_d = nc.dram_tensor("out", [BL, C3], f32, kind="ExternalOutput")

    with tile.TileContext(nc) as tc, ExitStack() as ctx:
        xpool = ctx.enter_context(tc.tile_pool(name="xq", bufs=KP))
        wpool = ctx.enter_context(tc.tile_pool(name="w", bufs=1))
        hpool = ctx.enter_context(tc.tile_pool(name="h", bufs=RF))
        hnpool = ctx.enter_context(tc.tile_pool(name="hn", bufs=RF))
        hsqpool = ctx.enter_context(tc.tile_pool(name="hsq", bufs=2))
        smallpool = ctx.enter_context(tc.tile_pool(name="small", bufs=1))
        psum_fc = ctx.enter_context(tc.tile_pool(name="psfc", bufs=8, space="PSUM"))
        drampool = ctx.enter_context(tc.tile_pool(name="dram", bufs=1, space="DRAM"))

        # ---- dummy AllReduce at t=0: entry-barrier + cc-stream warmup ----
        ar0src = smallpool.tile([1, 4], f32, tag="ar0src")
        nc.vector.memset(ar0src[:], 1.0)
        ar0_in = drampool.tile([1, 4], f32, tag="ar0in")
        ar0_out = drampool.tile([1, 4], f32, tag="ar0out")
        nc.sync.dma_start(ar0_in[:], ar0src[:])
        nc.gpsimd.collective_compute(
            "AllReduce",
            OP.add,
            replica_groups=[list(range(NCORES))],
            ins=[ar0_in[:].opt()],
            outs=[ar0_out[:].opt()],
        )

        # ---- loads: everything is a plain contiguous DMA, 2 queues ----
        # need-order interleave: (wfc pair kp, x pair kp piece A) feed matmul
        # pair kp of row-half 0; the B pieces feed row-half 1.
        cate_sb = smallpool.tile([128, NT], i32, tag="cate")
        nc.sync.dma_start(cate_sb[:], cat_d.ap())

        wfc_sb = wpool.tile([128, KP * RF * 2 * 128], fp8, tag="wfc")
        xts = [xpool.tile([128, 2 * BL], fp8, tag="xq", name=f"xq{k}") for k in range(KP)]

        def x3(ap):  # [128, 2*BL] -> [128, 2, BL]
            return ap.rearrange("p (j r) -> p j r", j=2)

        WCH = RF * 2 * 128  # 1024 wfc cols per pair
        for kp in range(KP):
            eng = nc.sync if kp % 2 == 0 else nc.scalar
            eng.dma_start(
                wfc_sb[:, kp * WCH : (kp + 1) * WCH],
                wfc_d.ap()[:, kp * WCH : (kp + 1) * WCH],
            )
            eng.dma_start(
                x3(xts[kp][:])[:, :, 0 : BL // 2],
                x3(xq_d.ap()[kp])[:, :, 0 : BL // 2],
            )
        for kp in range(KP):
            eng = nc.sync if kp % 2 == 0 else nc.scalar
            eng.dma_start(
                x3(xts[kp][:])[:, :, BL // 2 : BL],
                x3(xq_d.ap()[kp])[:, :, BL // 2 : BL],
            )

        gam_sb = smallpool.tile([128, RF], f32, tag="gam")
        nc.scalar.dma_start(gam_sb[:], gam_d.ap())
        bet_sb = smallpool.tile([128, RF], f32, tag="bet")
        nc.scalar.dma_start(bet_sb[:], bet_d.ap())
        bclf_sb = smallpool.tile([1, C3], bf16, tag="bclf")
        nc.scalar.dma_start(bclf_sb[:], bclf_d.ap())
        wclf_sb = wpool.tile([128, RF * C3], bf16, tag="wclf")
        nc.sync.dma_start(wclf_sb[:], wclf_d.ap())

        ones_sb = smallpool.tile([1, 128], bf16, tag="ones")
        nc.vector.memset(ones_sb[:], 1.0)
        eps_sb = smallpool.tile([128, 1], f32, tag="eps")
        nc.vector.memset(eps_sb[:], BN_EPS * WSCALE * WSCALE)

        # masked positions default to -100; gathers fill the keep-table
        outs_sb = smallpool.tile([128, NT * C3], f32, tag="outs")
        nc.gpsimd.memset(outs_sb[:], MASK_VAL)
        mask_sb = smallpool.tile([128, NT * C3], mybir.dt.uint8, tag="mask")
        for t in range(NT):
            nc.gpsimd.indirect_dma_start(
                out=mask_sb[:, t * C3 : (t + 1) * C3],
                out_offset=None,
                in_=m2_d.ap(),
                in_offset=bass.IndirectOffsetOnAxis(ap=cate_sb[:, t : t + 1], axis=0),
            )

        # ---- fc matmul (fp8 DoubleRow) + streaming BN stats ----
        h_sb = [hpool.tile([128, BL], bf16, tag="h", name=f"h{f}") for f in range(RF)]
        # stats col layout: r*8 + f = sum_h, r*8 + 4 + f = sum_h^2
        sums_sb = smallpool.tile([128, NRC * 8], f32, tag="sums")

        def wslice(kp, f):  # [128, 2, 128] stationary pair
            off = ((kp * RF + f) * 2) * 128
            return wfc_sb[:, off : off + 256].rearrange("p (j m) -> p j m", j=2)

        def drain(f, r, psum):
            nc.vector.tensor_scalar(
                out=h_sb[f][:, r * 512 : (r + 1) * 512],
                in0=psum[:],
                scalar1=1.0,
                scalar2=None,
                op0=OP.mult,
                op1=OP.add,
                accum_out=sums_sb[:, r * 8 + f : r * 8 + f + 1],
            )
            hsq = hsqpool.tile([128, 512], bf16, tag="hsq", name=f"hsq{f}_{r}")
            nc.scalar.activation(
                out=hsq[:],
                in_=psum[:],
                func=AF.Square,
                accum_out=sums_sb[:, r * 8 + 4 + f : r * 8 + 4 + f + 1],
            )

        # half 0 (rows 0..1023): kp-outer, paced by the x DMA stream
        psums0 = {
            (f, r): psum_fc.tile([128, 512], f32, tag="ps", name=f"psA{f}_{r}")
            for f in range(RF)
            for r in range(2)
        }
        for kp in range(KP):
            for f in range(RF):
                lhsT = wslice(kp, f)
                for r in range(2):
                    nc.tensor.matmul(
                        psums0[(f, r)][:],
                        lhsT=lhsT,
                        rhs=x3(xts[kp][:])[:, :, r * 512 : (r + 1) * 512],
                        start=(kp == 0),
                        stop=(kp == KP - 1),
                        perf_mode=DR,
                    )
        for f in range(RF):
            for r in range(2):
                drain(f, r, psums0[(f, r)])

        # half 1 (rows 1024..2047): f-outer so drains stagger behind the PE
        for f in range(RF):
            psums1 = {
                r: psum_fc.tile([128, 512], f32, tag="ps", name=f"psB{f}_{r}")
                for r in range(2, 4)
            }
            for kp in range(KP):
                lhsT = wslice(kp, f)
                for r in range(2, 4):
                    nc.tensor.matmul(
                        psums1[r][:],
                        lhsT=lhsT,
                        rhs=x3(xts[kp][:])[:, :, r * 512 : (r + 1) * 512],
                        start=(kp == 0),
                        stop=(kp == KP - 1),
                        perf_mode=DR,
                    )
            for r in range(2, 4):
                drain(f, r, psums1[r])

        # fold the 4 row-chunk partials: stats = sum_r sums[:, r*8:(r+1)*8]
        t01 = smallpool.tile([128, 8], f32, tag="t01")
        nc.vector.tensor_tensor(out=t01[:], in0=sums_sb[:, 0:8], in1=sums_sb[:, 8:16], op=OP.add)
        t23 = smallpool.tile([128, 8], f32, tag="t23")
        nc.vector.tensor_tensor(out=t23[:], in0=sums_sb[:, 16:24], in1=sums_sb[:, 24:32], op=OP.add)
        stats = smallpool.tile([128, 8], f32, tag="stats")
        nc.vector.tensor_tensor(out=stats[:], in0=t01[:], in1=t23[:], op=OP.add)

        # ---- stats AllReduce across the 8 cores ----
        cc_in = drampool.tile([128, 8], f32, tag="ccin")
        cc_out = drampool.tile([128, 8], f32, tag="ccout")
        nc.sync.dma_start(cc_in[:], stats[:])
        ar1 = nc.gpsimd.collective_compute(
            "AllReduce",
            OP.add,
            replica_groups=[list(range(NCORES))],
            ins=[cc_in[:].opt()],
            outs=[cc_out[:].opt()],
        )
        stats_all = smallpool.tile([128, 8], f32, tag="statsall")
        nc.scalar.dma_start(stats_all[:], cc_out[:])

        # PE warm-up during the AR window: HAM throttles an idle PE; a short
        # burst of dummy matmuls keeps the clf matmuls at full duty.
        warm_ps = psum_fc.tile([128, 512], f32, tag="ps", name="warmps")
        for wi in range(12):
            mi = nc.tensor.matmul(
                warm_ps[:],
                lhsT=h_sb[0][:, 0:128],
                rhs=h_sb[0][:, 0:512],
                start=True,
                stop=True,
                skip_group_check=True,
            )
            if wi == 0:
                tile.add_dep_helper(mi.ins, ar1.ins, sync=True, reason="warm PE during AR")

        # ---- BN consts: s = gamma*rsqrt(var+eps'), t = beta - mean*s ----
        # (scaled domain: stats are of h' = 64*h, eps' = eps*64^2; svec/tvec
        # come out in the h' domain so hn = relu(h'*s + t) is exact)
        moments = smallpool.tile([128, 8], f32, tag="moments")
        nc.vector.tensor_scalar_mul(moments[:], stats_all[:], 1.0 / B)
        msq = smallpool.tile([128, RF], f32, tag="msq")
        nc.vector.tensor_tensor(out=msq[:], in0=moments[:, 0:RF], in1=moments[:, 0:RF], op=OP.mult)
        var = smallpool.tile([128, RF], f32, tag="var")
        nc.vector.tensor_tensor(out=var[:], in0=moments[:, RF : 2 * RF], in1=msq[:], op=OP.subtract)
        std = smallpool.tile([128, RF], f32, tag="std")
        nc.scalar.activation(std[:], var[:], AF.Sqrt, bias=eps_sb[:, 0:1])
        rstd = smallpool.tile([128, RF], f32, tag="rstd")
        nc.vector.reciprocal(rstd[:], std[:])
        svec = smallpool.tile([128, RF], f32, tag="svec")
        nc.vector.tensor_tensor(out=svec[:], in0=gam_sb[:], in1=rstd[:], op=OP.mult)
        mstmp = smallpool.tile([128, RF], f32, tag="mstmp")
        nc.vector.tensor_tensor(out=mstmp[:], in0=moments[:, 0:RF], in1=svec[:], op=OP.mult)
        tvec = smallpool.tile([128, RF], f32, tag="tvec")
        nc.vector.tensor_tensor(out=tvec[:], in0=bet_sb[:], in1=mstmp[:], op=OP.subtract)

        # ---- BN apply + relu, then clf matmul + bias + mask + store ----
        # per 512-row chunk: f0/f1 on ACT, f2 on DVE, f3 on GpSimd; the clf
        # matmuls for the chunk's 4 row tiles follow, preds on DVE, stores
        # alternate the two DMA queues.
        hn_sb = [hnpool.tile([128, BL], bf16, tag="hn", name=f"hn{f}") for f in range(RF)]
        for c in range(NRC):
            cs = slice(c * 512, (c + 1) * 512)
            for f in (0, 1):
                nc.scalar.activation(
                    out=hn_sb[f][:, cs],
                    in_=h_sb[f][:, cs],
                    func=AF.Relu,
                    scale=svec[:, f : f + 1],
                    bias=tvec[:, f : f + 1],
                )
            for f, eng in ((2, nc.vector), (3, nc.gpsimd)):
                # tensor_scalar only takes an AP for scalar1, so the BN
                # affine + relu is a 3-op chain here (vs 1 ACT op on Scalar)
                eng.tensor_scalar(
                    out=hn_sb[f][:, cs],
                    in0=h_sb[f][:, cs],
                    scalar1=svec[:, f : f + 1],
                    scalar2=None,
                    op0=OP.mult,
                )
                eng.tensor_scalar(
                    out=hn_sb[f][:, cs],
                    in0=hn_sb[f][:, cs],
                    scalar1=tvec[:, f : f + 1],
                    scalar2=0.0,
                    op0=OP.add,
                    op1=OP.max,
                )
            for sub in range(4):
                t = c * 4 + sub
                po = psum_fc.tile([128, C3], f32, tag="ps", name=f"po{t}")
                for f in range(RF):
                    nc.tensor.matmul(
                        po[:],
                        lhsT=hn_sb[f][:, t * 128 : (t + 1) * 128],
                        rhs=wclf_sb[:, f * C3 : (f + 1) * C3],
                        start=(f == 0),
                        stop=False,
                    )
                nc.tensor.matmul(
                    po[:], lhsT=ones_sb[:], rhs=bclf_sb[:], start=False, stop=True
                )
                nc.vector.copy_predicated(
                    outs_sb[:, t * C3 : (t + 1) * C3],
                    mask_sb[:, t * C3 : (t + 1) * C3],
                    po[:],
                )
                eng = nc.sync if t % 2 == 0 else nc.scalar
                eng.dma_start(
                    out_d.ap()[t * 128 : (t + 1) * 128, :],
                    outs_sb[:, t * C3 : (t + 1) * C3],
                )

    nc.compile()
    return nc


def _get_nc():
    if "nc" not in _CACHE:
        _CACHE["nc"] = _build_nc()
    return _CACHE["nc"]


def make_in_maps(**inputs):
    """Host-side marshaling: shard/cast/layout the full inputs per core."""
    bf16 = ml_dtypes.bfloat16
    e4m3 = ml_dtypes.float8_e4m3  # IEEE variant, max 240 — matches TRN FP8_EXP4

    x = np.asarray(inputs["swem_vec"], dtype=np.float32)
    # x^T, pre-paired for DoubleRow: [KP, 128, 2*BL] with d = kp*256 + j*128 + p
    xT8 = np.ascontiguousarray(x.T).astype(e4m3)  # [D, B]

    wfc = np.asarray(inputs["W_fc"], dtype=np.float32) * WSCALE
    wfc8 = np.ascontiguousarray(
        wfc.reshape(KP, 2, 128, RF, 128).transpose(2, 0, 3, 1, 4).reshape(128, -1)
    ).astype(e4m3)

    wclf = np.asarray(inputs["W_clf"], dtype=np.float32)
    wclf_h = np.ascontiguousarray(
        wclf.reshape(RF, 128, C3).transpose(1, 0, 2).reshape(128, RF * C3)
    ).astype(bf16)
    bclf = np.asarray(inputs["b_clf"], dtype=np.float32).astype(bf16)[None, :]
    gam = np.ascontiguousarray(np.asarray(inputs["gamma"], dtype=np.float32).reshape(RF, 128).T)
    bet = np.ascontiguousarray(np.asarray(inputs["beta"], dtype=np.float32).reshape(RF, 128).T)
    m2 = (~np.asarray(inputs["mask2"])).astype(np.uint8)  # 1 = keep, 0 = mask to -100
    cate = np.asarray(inputs["cate2"]).astype(np.int32)

    in_maps = []
    for c in range(NCORES):
        sl = slice(c * BL, (c + 1) * BL)
        xc = xT8[:, sl]  # [D, BL]
        xq = np.ascontiguousarray(
            xc.reshape(KP, 2, 128, BL).transpose(0, 2, 1, 3).reshape(KP, 128, 2 * BL)
        )
        in_maps.append(
            {
                "xq": xq,
                "wfc": wfc8,
                "wclf": wclf_h,
                "bclf": bclf,
                "gam": gam,
                "bet": bet,
                "m2": m2,
                "cat": np.ascontiguousarray(cate[sl].reshape(NT, 128).T),
            }
        )
    return in_maps


def run(in_maps, trace=False, **kwargs):
    from concourse.bass_utils import run_bass_kernel_spmd

    nc = _get_nc()
    return run_bass_kernel_spmd(
        nc, in_maps, core_ids=list(range(NCORES)), trace=trace, **kwargs
    )


def kernel(**inputs) -> np.ndarray:
    in_maps = make_in_maps(**inputs)
    res = run(in_maps, trace=False)
    return np.concatenate([res.results[c]["out"] for c in range(NCORES)], axis=0)


# revision 10
# speedup vs baseline: 1.4487x; 1.4487x over previous
"""nn_Cate3Classifier — 8-core Trainium2 Bass kernel (fp8 DoubleRow rewrite).

Math (see reference):
    h   = swem_vec @ W_fc (+ b_fc)        # b_fc cancels inside BatchNorm
    hn  = relu((h - mu) * rsqrt(var + eps) * gamma + beta)   # batch stats over ALL B rows
    out = hn @ W_clf + b_clf
    out[i, j] = -100 where mask2[cate2[i], j]

Distribution: pure data-parallel over the batch (2048 rows/core on 8 cores).
Weights/masks replicated. BN batch statistics use one AllReduce of per-core
[sum_h, sum_h^2] (f32 [128, 8] = 4 KiB), preceded by a tiny dummy AllReduce
at kernel start that absorbs inter-core start skew and warms the cc stream.

Key speed choices vs the bf16 baseline:
  - fc matmul runs in fp8 e4m3 DoubleRow perf mode (2 contraction rows per
    cycle -> 2x PE throughput). W_fc is pre-scaled by 64 host-side so its
    values sit in e4m3's normal range; the scale cancels inside BatchNorm
    (eps is scaled by 64^2 to keep the math exact).
  - x is pre-transposed AND pre-paired host-side, so all device loads are
    plain contiguous DMAs (no HW DMA-transpose stream, no phase fencing).
  - single late stats AllReduce; dummy AR at t=0 hides the entry barrier.
  - BN apply + clf matmul + mask + store pipelined per 512-row chunk with
    the BN work split across Scalar/Vector/GpSimd engines.
"""

import numpy as np
import ml_dtypes

B, D, H, C3, C2 = 16384, 2048, 512, 125, 64
NCORES = 8
BL = B // NCORES      # 2048 rows per core
KP = D // 256         # 8 fp8 DoubleRow contraction pairs (2x128 each)
RF = H // 128         # 4 feature chunks
NRC = BL // 512       # 4 row chunks of 512
NT = BL // 128        # 16 output row tiles of 128
BN_EPS = 1e-5
MASK_VAL = -100.0
WSCALE = 64.0         # host pre-scale on W_fc; cancels in BN (eps scaled too)

_CACHE = {}


def _build_nc():
    from contextlib import ExitStack

    import concourse.bass as bass
    import concourse.tile as tile
    from concourse import bacc, mybir

    f32 = mybir.dt.float32
    bf16 = mybir.dt.bfloat16
    fp8 = mybir.dt.float8e4
    i32 = mybir.dt.int32
    AF = mybir.ActivationFunctionType
    OP = mybir.AluOpType
    DR = mybir.MatmulPerfMode.DoubleRow

    nc = bacc.Bacc("TRN2", target_bir_lowering=False, debug=False, num_devices=NCORES)

    xq_d = nc.dram_tensor("xq", [KP, 128, 2 * BL], fp8, kind="ExternalInput")
    wfc_d = nc.dram_tensor("wfc", [128, KP * RF * 2 * 128], fp8, kind="ExternalInput")
    wclf_d = nc.dram_tensor("wclf", [128, RF * C3], bf16, kind="ExternalInput")
    bclf_d = nc.dram_tensor("bclf", [1, C3], bf16, kind="ExternalInput")
    gam_d = nc.dram_tensor("gam", [128, RF], f32, kind="ExternalInput")
    bet_d = nc.dram_tensor("bet", [128, RF], f32, kind="ExternalInput")
    m2_d = nc.dram_tensor("m2", [C2, C3], mybir.dt.uint8, kind="ExternalInput")
    cat_d = nc.dram_tensor("cat", [128, NT], i32, kind="ExternalInput")
    out_d = nc.dram_tensor("out", [BL, C3], f32, kind="ExternalOutput")

    with tile.TileContext(nc) as tc, ExitStack() as ctx:
        xpool = ctx.enter_context(tc.tile_pool(name="xq", bufs=KP))
        wpool = ctx.enter_context(tc.tile_pool(name="w", bufs=1))
        hpool = ctx.enter_context(tc.tile_pool(name="h", bufs=RF))
        hnpool = ctx.enter_context(tc.tile_pool(name="hn", bufs=RF))
        hsqpool = ctx.enter_context(tc.tile_pool(name="hsq", bufs=2))
        smallpool = ctx.enter_context(tc.tile_pool(name="small", bufs=1))
        psum_fc = ctx.enter_context(tc.tile_pool(name="psfc", bufs=8, space="PSUM"))
        drampool = ctx.enter_context(tc.tile_pool(name="dram", bufs=1, space="DRAM"))

        # ---- loads: everything is a plain contiguous DMA, 2 queues ----
        # need-order interleave: (wfc pair kp, x pair kp piece A) feed matmul
        # pair kp of row-half 0; the B pieces feed row-half 1.
        cate_sb = smallpool.tile([128, NT], i32, tag="cate")
        nc.sync.dma_start(cate_sb[:], cat_d.ap())

        wfc_sb = wpool.tile([128, KP * RF * 2 * 128], fp8, tag="wfc")
        xts = [xpool.tile([128, 2 * BL], fp8, tag="xq", name=f"xq{k}") for k in range(KP)]

        def x3(ap):  # [128, 2*BL] -> [128, 2, BL]
            return ap.rearrange("p (j r) -> p j r", j=2)

        WCH = RF * 2 * 128  # 1024 wfc cols per pair
        for kp in range(KP):
            eng = nc.sync if kp % 2 == 0 else nc.scalar
            eng.dma_start(
                wfc_sb[:, kp * WCH : (kp + 1) * WCH],
                wfc_d.ap()[:, kp * WCH : (kp + 1) * WCH],
            )
            eng.dma_start(
                x3(xts[kp][:])[:, :, 0 : BL // 2],
                x3(xq_d.ap()[kp])[:, :, 0 : BL // 2],
            )
        for kp in range(KP):
            eng = nc.sync if kp % 2 == 0 else nc.scalar
            eng.dma_start(
                x3(xts[kp][:])[:, :, BL // 2 : BL],
                x3(xq_d.ap()[kp])[:, :, BL // 2 : BL],
            )

        gam_sb = smallpool.tile([128, RF], f32, tag="gam")
        nc.scalar.dma_start(gam_sb[:], gam_d.ap())
        bet_sb = smallpool.tile([128, RF], f32, tag="bet")
        nc.scalar.dma_start(bet_sb[:], bet_d.ap())
        bclf_sb = smallpool.tile([1, C3], bf16, tag="bclf")
        nc.scalar.dma_start(bclf_sb[:], bclf_d.ap())
        wclf_sb = wpool.tile([128, RF * C3], bf16, tag="wclf")
        nc.sync.dma_start(wclf_sb[:], wclf_d.ap())

        ones_sb = smallpool.tile([1, 128], bf16, tag="ones")
        nc.vector.memset(ones_sb[:], 1.0)
        eps_sb = smallpool.tile([128, 1], f32, tag="eps")
        nc.vector.memset(eps_sb[:], BN_EPS * WSCALE * WSCALE)

        # masked positions default to -100; gathers fill the keep-table
        outs_sb = smallpool.tile([128, NT * C3], f32, tag="outs")
        nc.gpsimd.memset(outs_sb[:], MASK_VAL)
        mask_sb = smallpool.tile([128, NT * C3], mybir.dt.uint8, tag="mask")
        for t in range(NT):
            nc.gpsimd.indirect_dma_start(
                out=mask_sb[:, t * C3 : (t + 1) * C3],
                out_offset=None,
                in_=m2_d.ap(),
                in_offset=bass.IndirectOffsetOnAxis(ap=cate_sb[:, t : t + 1], axis=0),
            )

        # ---- fc matmul (fp8 DoubleRow) + streaming BN stats ----
        h_sb = [hpool.tile([128, BL], bf16, tag="h", name=f"h{f}") for f in range(RF)]
        # stats col layout: r*8 + f = sum_h, r*8 + 4 + f = sum_h^2
        sums_sb = smallpool.tile([128, NRC * 8], f32, tag="sums")

        def wslice(kp, f):  # [128, 2, 128] stationary pair
            off = ((kp * RF + f) * 2) * 128
            return wfc_sb[:, off : off + 256].rearrange("p (j m) -> p j m", j=2)

        def drain(f, r, psum):
            nc.vector.tensor_scalar(
                out=h_sb[f][:, r * 512 : (r + 1) * 512],
                in0=psum[:],
                scalar1=1.0,
                scalar2=None,
                op0=OP.mult,
                op1=OP.add,
                accum_out=sums_sb[:, r * 8 + f : r * 8 + f + 1],
            )
            hsq = hsqpool.tile([128, 512], bf16, tag="hsq", name=f"hsq{f}_{r}")
            nc.scalar.activation(
                out=hsq[:],
                in_=psum[:],
                func=AF.Square,
                accum_out=sums_sb[:, r * 8 + 4 + f : r * 8 + 4 + f + 1],
            )

        # half 0 (rows 0..1023): kp-outer, paced by the x DMA stream
        psums0 = {
            (f, r): psum_fc.tile([128, 512], f32, tag="ps", name=f"psA{f}_{r}")
            for f in range(RF)
            for r in range(2)
        }
        for kp in range(KP):
            for f in range(RF):
                lhsT = wslice(kp, f)
                for r in range(2):
                    nc.tensor.matmul(
                        psums0[(f, r)][:],
                        lhsT=lhsT,
                        rhs=x3(xts[kp][:])[:, :, r * 512 : (r + 1) * 512],
                        start=(kp == 0),
                        stop=(kp == KP - 1),
                        perf_mode=DR,
                    )
        for f in range(RF):
            for r in range(2):
                drain(f, r, psums0[(f, r)])
        # fold half-0 row-chunk partials while half-1 computes
        t01 = smallpool.tile([128, 8], f32, tag="t01")
        nc.vector.tensor_tensor(out=t01[:], in0=sums_sb[:, 0:8], in1=sums_sb[:, 8:16], op=OP.add)

        # half 1 (rows 1024..2047): f-outer so drains stagger behind the PE
        for f in range(RF):
            psums1 = {
                r: psum_fc.tile([128, 512], f32, tag="ps", name=f"psB{f}_{r}")
                for r in range(2, 4)
            }
            for kp in range(KP):
                lhsT = wslice(kp, f)
                for r in range(2, 4):
                    nc.tensor.matmul(
                        psums1[r][:],
                        lhsT=lhsT,
                        rhs=x3(xts[kp][:])[:, :, r * 512 : (r + 1) * 512],
                        start=(kp == 0),
                        stop=(kp == KP - 1),
                        perf_mode=DR,
                    )
            for r in range(2, 4):
                drain(f, r, psums1[r])

        # fold the remaining row-chunk partials
        t23 = smallpool.tile([128, 8], f32, tag="t23")
        nc.vector.tensor_tensor(out=t23[:], in0=sums_sb[:, 16:24], in1=sums_sb[:, 24:32], op=OP.add)
        stats = smallpool.tile([128, 8], f32, tag="stats")
        nc.vector.tensor_tensor(out=stats[:], in0=t01[:], in1=t23[:], op=OP.add)

        # ---- stats AllReduce across the 8 cores ----
        cc_in = drampool.tile([128, 8], f32, tag="ccin")
        cc_out = drampool.tile([128, 8], f32, tag="ccout")
        ccdma = nc.sync.dma_start(cc_in[:], stats[:])
        ar1 = nc.gpsimd.collective_compute(
            "AllReduce",
            OP.add,
            replica_groups=[list(range(NCORES))],
            ins=[cc_in[:].opt()],
            outs=[cc_out[:].opt()],
        )
        stats_all = smallpool.tile([128, 8], f32, tag="statsall")
        nc.scalar.dma_start(stats_all[:], cc_out[:])

        # PE warm-up during the AR wait: HAM throttles an idle PE; a burst of
        # dummy matmuls right after fc (gated on the local stats DMA, NOT the
        # collective) keeps duty up for the clf matmuls.
        warm_ps = psum_fc.tile([128, 512], f32, tag="ps", name="warmps")
        for wi in range(24):
            mi = nc.tensor.matmul(
                warm_ps[:],
                lhsT=h_sb[0][:, 0:128],
                rhs=h_sb[0][:, 0:512],
                start=True,
                stop=True,
                skip_group_check=True,
            )
            if wi == 0:
                tile.add_dep_helper(mi.ins, ccdma.ins, sync=True, reason="warm PE during AR")

        # ---- BN consts: s = gamma*rsqrt(var+eps'), t = beta - mean*s ----
        # (scaled domain: stats are of h' = 64*h, eps' = eps*64^2; svec/tvec
        # come out in the h' domain so hn = relu(h'*s + t) is exact)
        moments = smallpool.tile([128, 8], f32, tag="moments")
        nc.vector.tensor_scalar_mul(moments[:], stats_all[:], 1.0 / B)
        msq = smallpool.tile([128, RF], f32, tag="msq")
        nc.vector.tensor_tensor(out=msq[:], in0=moments[:, 0:RF], in1=moments[:, 0:RF], op=OP.mult)
        var = smallpool.tile([128, RF], f32, tag="var")
        nc.vector.tensor_tensor(out=var[:], in0=moments[:, RF : 2 * RF], in1=msq[:], op=OP.subtract)
        std = smallpool.tile([128, RF], f32, tag="std")
        nc.scalar.activation(std[:], var[:], AF.Sqrt, bias=eps_sb[:, 0:1])
        rstd = smallpool.tile([128, RF], f32, tag="rstd")
        nc.vector.reciprocal(rstd[:], std[:])
        svec = smallpool.tile([128, RF], f32, tag="svec")
        nc.vector.tensor_tensor(out=svec[:], in0=gam_sb[:], in1=rstd[:], op=OP.mult)
        mstmp = smallpool.tile([128, RF], f32, tag="mstmp")
        nc.vector.tensor_tensor(out=mstmp[:], in0=moments[:, 0:RF], in1=svec[:], op=OP.mult)
        tvec = smallpool.tile([128, RF], f32, tag="tvec")
        nc.vector.tensor_tensor(out=tvec[:], in0=bet_sb[:], in1=mstmp[:], op=OP.subtract)

        # ---- BN apply + relu, then clf matmul + bias + mask + store ----
        # BN apply runs entirely on ACT (per-partition scale/bias is native
        # and fast there; DVE/GpSimd tensor_scalar with an AP scalar is a
        # ~10x slow path). 1024-col chunks amortize ACT op overhead; the
        # chunk's 8 row tiles of clf matmuls + preds + stores pipeline after.
        hn_sb = [hnpool.tile([128, BL], bf16, tag="hn", name=f"hn{f}") for f in range(RF)]
        for c in range(2):
            cs = slice(c * 1024, (c + 1) * 1024)
            for f in range(RF):
                nc.scalar.activation(
                    out=hn_sb[f][:, cs],
                    in_=h_sb[f][:, cs],
                    func=AF.Relu,
                    scale=svec[:, f : f + 1],
                    bias=tvec[:, f : f + 1],
                )
            for sub in range(8):
                t = c * 8 + sub
                po = psum_fc.tile([128, C3], f32, tag="ps", name=f"po{t}")
                for f in range(RF):
                    nc.tensor.matmul(
                        po[:],
                        lhsT=hn_sb[f][:, t * 128 : (t + 1) * 128],
                        rhs=wclf_sb[:, f * C3 : (f + 1) * C3],
                        start=(f == 0),
                        stop=False,
                    )
                nc.tensor.matmul(
                    po[:], lhsT=ones_sb[:], rhs=bclf_sb[:], start=False, stop=True
                )
                nc.vector.copy_predicated(
                    outs_sb[:, t * C3 : (t + 1) * C3],
                    mask_sb[:, t * C3 : (t + 1) * C3],
                    po[:],
                )
                eng = nc.sync if t % 2 == 0 else nc.scalar
                eng.dma_start(
                    out_d.ap()[t * 128 : (t + 1) * 128, :],
                    outs_sb[:, t * C3 : (t + 1) * C3],
                )

    nc.compile()
    return nc


def _get_nc():
    if "nc" not in _CACHE:
        _CACHE["nc"] = _build_nc()
    return _CACHE["nc"]


def make_in_maps(**inputs):
    """Host-side marshaling: shard/cast/layout the full inputs per core."""
    bf16 = ml_dtypes.bfloat16
    e4m3 = ml_dtypes.float8_e4m3  # IEEE variant, max 240 — matches TRN FP8_EXP4

    x = np.asarray(inputs["swem_vec"], dtype=np.float32)
    # x^T, pre-paired for DoubleRow: [KP, 128, 2*BL] with d = kp*256 + j*128 + p
    xT8 = np.ascontiguousarray(x.T).astype(e4m3)  # [D, B]

    wfc = np.asarray(inputs["W_fc"], dtype=np.float32) * WSCALE
    wfc8 = np.ascontiguousarray(
        wfc.reshape(KP, 2, 128, RF, 128).transpose(2, 0, 3, 1, 4).reshape(128, -1)
    ).astype(e4m3)

    wclf = np.asarray(inputs["W_clf"], dtype=np.float32)
    wclf_h = np.ascontiguousarray(
        wclf.reshape(RF, 128, C3).transpose(1, 0, 2).reshape(128, RF * C3)
    ).astype(bf16)
    bclf = np.asarray(inputs["b_clf"], dtype=np.float32).astype(bf16)[None, :]
    gam = np.ascontiguousarray(np.asarray(inputs["gamma"], dtype=np.float32).reshape(RF, 128).T)
    bet = np.ascontiguousarray(np.asarray(inputs["beta"], dtype=np.float32).reshape(RF, 128).T)
    m2 = (~np.asarray(inputs["mask2"])).astype(np.uint8)  # 1 = keep, 0 = mask to -100
    cate = np.asarray(inputs["cate2"]).astype(np.int32)

    in_maps = []
    for c in range(NCORES):
        sl = slice(c * BL, (c + 1) * BL)
        xc = xT8[:, sl]  # [D, BL]
        xq = np.ascontiguousarray(
            xc.reshape(KP, 2, 128, BL).transpose(0, 2, 1, 3).reshape(KP, 128, 2 * BL)
        )
        in_maps.append(
            {
                "xq": xq,
                "wfc": wfc8,
                "wclf": wclf_h,
                "bclf": bclf,
                "gam": gam,
                "bet": bet,
                "m2": m2,
                "cat": np.ascontiguousarray(cate[sl].reshape(NT, 128).T),
            }
        )
    return in_maps


def run(in_maps, trace=False, **kwargs):
    from concourse.bass_utils import run_bass_kernel_spmd

    nc = _get_nc()
    return run_bass_kernel_spmd(
        nc, in_maps, core_ids=list(range(NCORES)), trace=trace, **kwargs
    )


def kernel(**inputs) -> np.ndarray:
    in_maps = make_in_maps(**inputs)
    res = run(in_maps, trace=False)
    return np.concatenate([res.results[c]["out"] for c in range(NCORES)], axis=0)


# revision 19
# speedup vs baseline: 1.5091x; 1.0417x over previous
"""nn_Cate3Classifier — 8-core Trainium2 Bass kernel (fp8 DoubleRow rewrite).

Math (see reference):
    h   = swem_vec @ W_fc (+ b_fc)        # b_fc cancels inside BatchNorm
    hn  = relu((h - mu) * rsqrt(var + eps) * gamma + beta)   # batch stats over ALL B rows
    out = hn @ W_clf + b_clf
    out[i, j] = -100 where mask2[cate2[i], j]

Distribution: pure data-parallel over the batch (2048 rows/core on 8 cores).
Weights/masks replicated. BN batch statistics use one AllReduce of per-core
[sum_h, sum_h^2] (f32 [128, 8] = 4 KiB), preceded by a tiny dummy AllReduce
at kernel start that absorbs inter-core start skew and warms the cc stream.

Key speed choices vs the bf16 baseline:
  - fc matmul runs in fp8 e4m3 DoubleRow perf mode (2 contraction rows per
    cycle -> 2x PE throughput). W_fc is pre-scaled by 64 host-side so its
    values sit in e4m3's normal range; the scale cancels inside BatchNorm
    (eps is scaled by 64^2 to keep the math exact).
  - x is pre-transposed AND pre-paired host-side, so all device loads are
    plain contiguous DMAs (no HW DMA-transpose stream, no phase fencing).
  - single late stats AllReduce; dummy AR at t=0 hides the entry barrier.
  - BN apply + clf matmul + mask + store pipelined per 512-row chunk with
    the BN work split across Scalar/Vector/GpSimd engines.
"""

import numpy as np
import ml_dtypes

B, D, H, C3, C2 = 16384, 2048, 512, 125, 64
NCORES = 8
BL = B // NCORES      # 2048 rows per core
KP = D // 256         # 8 fp8 DoubleRow contraction pairs (2x128 each)
RF = H // 128         # 4 feature chunks
NRC = BL // 512       # 4 row chunks of 512
NT = BL // 128        # 16 output row tiles of 128
BN_EPS = 1e-5
MASK_VAL = -100.0
WSCALE = 64.0         # host pre-scale on W_fc; cancels in BN (eps scaled too)

_CACHE = {}


def _build_nc():
    from contextlib import ExitStack

    import concourse.bass as bass
    import concourse.tile as tile
    from concourse import bacc, mybir

    f32 = mybir.dt.float32
    bf16 = mybir.dt.bfloat16
    fp8 = mybir.dt.float8e4
    i32 = mybir.dt.int32
    AF = mybir.ActivationFunctionType
    OP = mybir.AluOpType
    DR = mybir.MatmulPerfMode.DoubleRow

    nc = bacc.Bacc("TRN2", target_bir_lowering=False, debug=False, num_devices=NCORES)

    xq_d = nc.dram_tensor("xq", [KP, 128, 2 * BL], fp8, kind="ExternalInput")
    wfc_d = nc.dram_tensor("wfc", [128, KP * RF * 2 * 128], fp8, kind="ExternalInput")
    wclf_d = nc.dram_tensor("wclf", [128, RF * C3], bf16, kind="ExternalInput")
    bclf_d = nc.dram_tensor("bclf", [1, C3], bf16, kind="ExternalInput")
    gam_d = nc.dram_tensor("gam", [128, RF], f32, kind="ExternalInput")
    bet_d = nc.dram_tensor("bet", [128, RF], f32, kind="ExternalInput")
    m2_d = nc.dram_tensor("m2", [C2, C3], mybir.dt.uint8, kind="ExternalInput")
    cat_d = nc.dram_tensor("cat", [128, NT], i32, kind="ExternalInput")
    # partition-major output: out[p, t*C3+c] = row t*128+p; host unshuffles.
    # (row-major stores are 128x 500B strided writes per tile — ~90 GB/s —
    # while this layout stores as a few fully-linear DMAs at full BW)
    out_d = nc.dram_tensor("out", [128, NT * C3], f32, kind="ExternalOutput")

    with tile.TileContext(nc) as tc, ExitStack() as ctx:
        xpool = ctx.enter_context(tc.tile_pool(name="xq", bufs=KP))
        wpool = ctx.enter_context(tc.tile_pool(name="w", bufs=1))
        hpool = ctx.enter_context(tc.tile_pool(name="h", bufs=RF))
        hnpool = ctx.enter_context(tc.tile_pool(name="hn", bufs=RF))
        hsqpool = ctx.enter_context(tc.tile_pool(name="hsq", bufs=2))
        smallpool = ctx.enter_context(tc.tile_pool(name="small", bufs=1))
        psum_fc = ctx.enter_context(tc.tile_pool(name="psfc", bufs=8, space="PSUM"))
        drampool = ctx.enter_context(tc.tile_pool(name="dram", bufs=1, space="DRAM"))

        # ---- loads: everything is a plain contiguous DMA, 2 queues ----
        # need-order interleave: (wfc pair kp, x pair kp piece A) feed matmul
        # pair kp of row-half 0; the B pieces feed row-half 1.
        cate_sb = smallpool.tile([128, NT], i32, tag="cate")
        nc.sync.dma_start(cate_sb[:], cat_d.ap())

        wfc_sb = wpool.tile([128, KP * RF * 2 * 128], fp8, tag="wfc")
        xts = [xpool.tile([128, 2 * BL], fp8, tag="xq", name=f"xq{k}") for k in range(KP)]

        def x3(ap):  # [128, 2*BL] -> [128, 2, BL]
            return ap.rearrange("p (j r) -> p j r", j=2)

        WCH = RF * 2 * 128  # 1024 wfc cols per pair
        for kp in range(KP):
            eng = nc.sync if kp % 2 == 0 else nc.scalar
            eng.dma_start(
                wfc_sb[:, kp * WCH : (kp + 1) * WCH],
                wfc_d.ap()[:, kp * WCH : (kp + 1) * WCH],
            )
            eng.dma_start(
                x3(xts[kp][:])[:, :, 0 : BL // 2],
                x3(xq_d.ap()[kp])[:, :, 0 : BL // 2],
            )
        for kp in range(KP):
            eng = nc.sync if kp % 2 == 0 else nc.scalar
            eng.dma_start(
                x3(xts[kp][:])[:, :, BL // 2 : BL],
                x3(xq_d.ap()[kp])[:, :, BL // 2 : BL],
            )

        gam_sb = smallpool.tile([128, RF], f32, tag="gam")
        nc.scalar.dma_start(gam_sb[:], gam_d.ap())
        bet_sb = smallpool.tile([128, RF], f32, tag="bet")
        nc.scalar.dma_start(bet_sb[:], bet_d.ap())
        bclf_sb = smallpool.tile([1, C3], bf16, tag="bclf")
        nc.scalar.dma_start(bclf_sb[:], bclf_d.ap())
        wclf_sb = wpool.tile([128, RF * C3], bf16, tag="wclf")
        nc.sync.dma_start(wclf_sb[:], wclf_d.ap())

        ones_sb = smallpool.tile([1, 128], bf16, tag="ones")
        nc.vector.memset(ones_sb[:], 1.0)
        eps_sb = smallpool.tile([128, 1], f32, tag="eps")
        nc.vector.memset(eps_sb[:], BN_EPS * WSCALE * WSCALE)

        # masked positions default to -100; gathers fill the keep-table
        outs_sb = smallpool.tile([128, NT * C3], f32, tag="outs")
        nc.gpsimd.memset(outs_sb[:], MASK_VAL)
        mask_sb = smallpool.tile([128, NT * C3], mybir.dt.uint8, tag="mask")
        for t in range(NT):
            nc.gpsimd.indirect_dma_start(
                out=mask_sb[:, t * C3 : (t + 1) * C3],
                out_offset=None,
                in_=m2_d.ap(),
                in_offset=bass.IndirectOffsetOnAxis(ap=cate_sb[:, t : t + 1], axis=0),
            )

        # ---- fc matmul (fp8 DoubleRow) + streaming BN stats ----
        h_sb = [hpool.tile([128, BL], bf16, tag="h", name=f"h{f}") for f in range(RF)]
        # stats col layout: r*8 + f = sum_h, r*8 + 4 + f = sum_h^2
        sums_sb = smallpool.tile([128, NRC * 8], f32, tag="sums")

        def wslice(kp, f):  # [128, 2, 128] stationary pair
            off = ((kp * RF + f) * 2) * 128
            return wfc_sb[:, off : off + 256].rearrange("p (j m) -> p j m", j=2)

        def drain(f, r, psum):
            nc.vector.tensor_scalar(
                out=h_sb[f][:, r * 512 : (r + 1) * 512],
                in0=psum[:],
                scalar1=1.0,
                scalar2=None,
                op0=OP.mult,
                op1=OP.add,
                accum_out=sums_sb[:, r * 8 + f : r * 8 + f + 1],
            )
            hsq = hsqpool.tile([128, 512], bf16, tag="hsq", name=f"hsq{f}_{r}")
            nc.scalar.activation(
                out=hsq[:],
                in_=psum[:],
                func=AF.Square,
                accum_out=sums_sb[:, r * 8 + 4 + f : r * 8 + 4 + f + 1],
            )

        # half 0 (rows 0..1023): kp-outer, paced by the x DMA stream
        psums0 = {
            (f, r): psum_fc.tile([128, 512], f32, tag="ps", name=f"psA{f}_{r}")
            for f in range(RF)
            for r in range(2)
        }
        for kp in range(KP):
            for f in range(RF):
                lhsT = wslice(kp, f)
                for r in range(2):
                    nc.tensor.matmul(
                        psums0[(f, r)][:],
                        lhsT=lhsT,
                        rhs=x3(xts[kp][:])[:, :, r * 512 : (r + 1) * 512],
                        start=(kp == 0),
                        stop=(kp == KP - 1),
                        perf_mode=DR,
                    )
        for f in range(RF):
            for r in range(2):
                drain(f, r, psums0[(f, r)])
        # fold half-0 row-chunk partials while half-1 computes
        t01 = smallpool.tile([128, 8], f32, tag="t01")
        nc.vector.tensor_tensor(out=t01[:], in0=sums_sb[:, 0:8], in1=sums_sb[:, 8:16], op=OP.add)

        # half 1 (rows 1024..2047): f-outer so drains stagger behind the PE
        for f in range(RF):
            psums1 = {
                r: psum_fc.tile([128, 512], f32, tag="ps", name=f"psB{f}_{r}")
                for r in range(2, 4)
            }
            for kp in range(KP):
                lhsT = wslice(kp, f)
                for r in range(2, 4):
                    nc.tensor.matmul(
                        psums1[r][:],
                        lhsT=lhsT,
                        rhs=x3(xts[kp][:])[:, :, r * 512 : (r + 1) * 512],
                        start=(kp == 0),
                        stop=(kp == KP - 1),
                        perf_mode=DR,
                    )
            for r in range(2, 4):
                drain(f, r, psums1[r])

        # fold the remaining row-chunk partials
        t23 = smallpool.tile([128, 8], f32, tag="t23")
        nc.vector.tensor_tensor(out=t23[:], in0=sums_sb[:, 16:24], in1=sums_sb[:, 24:32], op=OP.add)
        stats = smallpool.tile([128, 8], f32, tag="stats")
        nc.vector.tensor_tensor(out=stats[:], in0=t01[:], in1=t23[:], op=OP.add)

        # ---- stats AllGather across the 8 cores + local fold ----
        # (measured ncfw floors on 8 cores: AllGather ~4.6us vs AllReduce
        # ~9.7us; the 3-op local tree-fold is well under the difference)
        cc_in = drampool.tile([128, 8], f32, tag="ccin")
        cc_out = drampool.tile([NCORES, 128, 8], f32, tag="ccout")
        ccdma = nc.sync.dma_start(cc_in[:], stats[:])
        ar1 = nc.gpsimd.collective_compute(
            "AllGather",
            OP.bypass,
            replica_groups=[list(range(NCORES))],
            ins=[cc_in[:].opt()],
            outs=[cc_out[:].opt()],
        )
        gath = smallpool.tile([128, 8 * NCORES], f32, tag="gath")
        nc.scalar.dma_start(
            gath[:].rearrange("p (r c) -> p r c", r=NCORES),
            cc_out[:].rearrange("r p c -> p r c"),
        )
        g1 = smallpool.tile([128, 32], f32, tag="g1")
        nc.vector.tensor_tensor(out=g1[:], in0=gath[:, 0:32], in1=gath[:, 32:64], op=OP.add)
        g2 = smallpool.tile([128, 16], f32, tag="g2")
        nc.vector.tensor_tensor(out=g2[:], in0=g1[:, 0:16], in1=g1[:, 16:32], op=OP.add)
        stats_all = smallpool.tile([128, 8], f32, tag="statsall")
        nc.vector.tensor_tensor(out=stats_all[:], in0=g2[:, 0:8], in1=g2[:, 8:16], op=OP.add)

        # PE warm-up during the AR wait: HAM throttles an idle PE; a burst of
        # dummy matmuls right after fc (gated on the local stats DMA, NOT the
        # collective) keeps duty up for the clf matmuls.
        warm_ps = psum_fc.tile([128, 512], f32, tag="ps", name="warmps")
        for wi in range(24):
            mi = nc.tensor.matmul(
                warm_ps[:],
                lhsT=h_sb[0][:, 0:128],
                rhs=h_sb[0][:, 0:512],
                start=True,
                stop=True,
                skip_group_check=True,
            )
            if wi == 0:
                tile.add_dep_helper(mi.ins, ccdma.ins, sync=True, reason="warm PE during AR")

        # ---- BN consts: s = gamma*rsqrt(var+eps'), t = beta - mean*s ----
        # (scaled domain: stats are of h' = 64*h, eps' = eps*64^2; svec/tvec
        # come out in the h' domain so hn = relu(h'*s + t) is exact)
        moments = smallpool.tile([128, 8], f32, tag="moments")
        nc.vector.tensor_scalar_mul(moments[:], stats_all[:], 1.0 / B)
        msq = smallpool.tile([128, RF], f32, tag="msq")
        nc.vector.tensor_tensor(out=msq[:], in0=moments[:, 0:RF], in1=moments[:, 0:RF], op=OP.mult)
        var = smallpool.tile([128, RF], f32, tag="var")
        nc.vector.tensor_tensor(out=var[:], in0=moments[:, RF : 2 * RF], in1=msq[:], op=OP.subtract)
        std = smallpool.tile([128, RF], f32, tag="std")
        nc.scalar.activation(std[:], var[:], AF.Sqrt, bias=eps_sb[:, 0:1])
        rstd = smallpool.tile([128, RF], f32, tag="rstd")
        nc.vector.reciprocal(rstd[:], std[:])
        svec = smallpool.tile([128, RF], f32, tag="svec")
        nc.vector.tensor_tensor(out=svec[:], in0=gam_sb[:], in1=rstd[:], op=OP.mult)
        mstmp = smallpool.tile([128, RF], f32, tag="mstmp")
        nc.vector.tensor_tensor(out=mstmp[:], in0=moments[:, 0:RF], in1=svec[:], op=OP.mult)
        tvec = smallpool.tile([128, RF], f32, tag="tvec")
        nc.vector.tensor_tensor(out=tvec[:], in0=bet_sb[:], in1=mstmp[:], op=OP.subtract)

        # ---- BN apply + relu, then clf matmul + bias + mask + store ----
        # BN apply runs entirely on ACT (per-partition scale/bias is native
        # and fast there; DVE/GpSimd tensor_scalar with an AP scalar is a
        # ~10x slow path). 1024-col chunks amortize ACT op overhead; the
        # chunk's 8 row tiles of clf matmuls + preds + stores pipeline after.
        hn_sb = [hnpool.tile([128, BL], bf16, tag="hn", name=f"hn{f}") for f in range(RF)]
        for c in range(2):
            cs = slice(c * 1024, (c + 1) * 1024)
            for f in range(RF):
                nc.scalar.activation(
                    out=hn_sb[f][:, cs],
                    in_=h_sb[f][:, cs],
                    func=AF.Relu,
                    scale=svec[:, f : f + 1],
                    bias=tvec[:, f : f + 1],
                )
            for sub4 in range(2):
                for sub in range(4):
                    t = c * 8 + sub4 * 4 + sub
                    po = psum_fc.tile([128, C3], f32, tag="ps", name=f"po{t}")
                    for f in range(RF):
                        nc.tensor.matmul(
                            po[:],
                            lhsT=hn_sb[f][:, t * 128 : (t + 1) * 128],
                            rhs=wclf_sb[:, f * C3 : (f + 1) * C3],
                            start=(f == 0),
                            stop=False,
                        )
                    nc.tensor.matmul(
                        po[:], lhsT=ones_sb[:], rhs=bclf_sb[:], start=False, stop=True
                    )
                    nc.vector.copy_predicated(
                        outs_sb[:, t * C3 : (t + 1) * C3],
                        mask_sb[:, t * C3 : (t + 1) * C3],
                        po[:],
                    )
                # one fully-linear store per 4-tile group (2KB/partition)
                t0 = c * 8 + sub4 * 4
                gs = slice(t0 * C3, (t0 + 4) * C3)
                eng = nc.sync if (c * 2 + sub4) % 2 == 0 else nc.gpsimd
                eng.dma_start(out_d.ap()[:, gs], outs_sb[:, gs])

    nc.compile()
    return nc


def _get_nc():
    if "nc" not in _CACHE:
        _CACHE["nc"] = _build_nc()
    return _CACHE["nc"]


def make_in_maps(**inputs):
    """Host-side marshaling: shard/cast/layout the full inputs per core."""
    bf16 = ml_dtypes.bfloat16
    e4m3 = ml_dtypes.float8_e4m3  # IEEE variant, max 240 — matches TRN FP8_EXP4

    x = np.asarray(inputs["swem_vec"], dtype=np.float32)
    # x^T, pre-paired for DoubleRow: [KP, 128, 2*BL] with d = kp*256 + j*128 + p
    xT8 = np.ascontiguousarray(x.T).astype(e4m3)  # [D, B]

    wfc = np.asarray(inputs["W_fc"], dtype=np.float32) * WSCALE
    wfc8 = np.ascontiguousarray(
        wfc.reshape(KP, 2, 128, RF, 128).transpose(2, 0, 3, 1, 4).reshape(128, -1)
    ).astype(e4m3)

    wclf = np.asarray(inputs["W_clf"], dtype=np.float32)
    wclf_h = np.ascontiguousarray(
        wclf.reshape(RF, 128, C3).transpose(1, 0, 2).reshape(128, RF * C3)
    ).astype(bf16)
    bclf = np.asarray(inputs["b_clf"], dtype=np.float32).astype(bf16)[None, :]
    gam = np.ascontiguousarray(np.asarray(inputs["gamma"], dtype=np.float32).reshape(RF, 128).T)
    bet = np.ascontiguousarray(np.asarray(inputs["beta"], dtype=np.float32).reshape(RF, 128).T)
    m2 = (~np.asarray(inputs["mask2"])).astype(np.uint8)  # 1 = keep, 0 = mask to -100
    cate = np.asarray(inputs["cate2"]).astype(np.int32)

    in_maps = []
    for c in range(NCORES):
        sl = slice(c * BL, (c + 1) * BL)
        xc = xT8[:, sl]  # [D, BL]
        xq = np.ascontiguousarray(
            xc.reshape(KP, 2, 128, BL).transpose(0, 2, 1, 3).reshape(KP, 128, 2 * BL)
        )
        in_maps.append(
            {
                "xq": xq,
                "wfc": wfc8,
                "wclf": wclf_h,
                "bclf": bclf,
                "gam": gam,
                "bet": bet,
                "m2": m2,
                "cat": np.ascontiguousarray(cate[sl].reshape(NT, 128).T),
            }
        )
    return in_maps


def run(in_maps, trace=False, **kwargs):
    from concourse.bass_utils import run_bass_kernel_spmd

    nc = _get_nc()
    return run_bass_kernel_spmd(
        nc, in_maps, core_ids=list(range(NCORES)), trace=trace, **kwargs
    )


def unshard(res) -> np.ndarray:
    # device output is partition-major [128, NT*C3]; unshuffle to [BL, C3]
    return np.concatenate(
        [
            res.results[c]["out"].reshape(128, NT, C3).transpose(1, 0, 2).reshape(BL, C3)
            for c in range(NCORES)
        ],
        axis=0,
    )


def kernel(**inputs) -> np.ndarray:
    in_maps = make_in_maps(**inputs)
    return unshard(run(in_maps, trace=False))
